# revision 11
# baseline (speedup 1.0000x reference)
"""Trainium2 Bass kernel for nn_EnhancedPGAT_CrossAttn_Layer (GNN message
passing, 2 hetero conv layers + layernorm).

Strategy (8 NeuronCores, SPMD single NEFF):
- Edges partitioned by TARGET range: core c owns targets [c*2500,(c+1)*2500).
  All segment stats / aggregation are core-local (no cross-core reduce of
  [N,*] tensors). Host sorts each core's edges by local target and packs
  them into 128-target "windows", padding each window's edge list to a
  multiple of 128 (uniform group structure across cores = one NEFF).
- Node-phase: per-node projections computed once on PE (bf16), packed into
  DRAM tables; per-edge data via SWDGE dma_gather (descriptor gather).
- Per-edge compute (dots, MLP, sigmoids) on DVE/ACT with edges on
  partitions, features on the free dim.
- Global edge softmax (softmax over ALL edges per head): local max/sum +
  AllGather collectives (32B), combined on-device.
- Segment softmax: |scores*ew| ~ 1e-6 so exp() is stable without the
  segment-max shift; a_e = exp(w_e)/d_t and the 1/d_t factors OUT of the
  V-aggregation -> aggregate u_t = sum(e4*v), d_t = sum(e4) and divide once
  per target. All segment sums via one-hot matmuls on PE accumulating in
  PSUM (edge windows are target-aligned so one-hot fits [128e,128t]).
- conv1 -> LN -> AllGather (bf16) of the updated trans features -> conv2.
"""
import sys

sys.path.insert(0, '/opt/trn_rl_repo')

from contextlib import ExitStack

import numpy as np
import ml_dtypes

import os

import concourse.bacc as bacc
import concourse.mybir as mybir
import concourse.tile as tile
from concourse.masks import make_identity

F32 = mybir.dt.float32
BF16 = mybir.dt.bfloat16
I16 = mybir.dt.int16
AF = mybir.ActivationFunctionType
ALU = mybir.AluOpType
BF16NP = ml_dtypes.bfloat16

NCORES = 8
D = 128
H = 4
HD = 32


# ----------------------------------------------------------------------------
# host-side packing helpers
# ----------------------------------------------------------------------------

def _wrap16(v):
    """[E] -> [128, E/16] int16; idx i at [i%16, i//16], replicated on the
    8 Q7 16-partition stripes."""
    blk = v.reshape(-1, 16).T.astype(np.int16)
    return np.tile(blk, (8, 1))


def _slotmaj(v, dtype):
    """[E] -> [128, E/128]; edge i at [i%128, i//128] (dma_gather layout)."""
    return np.ascontiguousarray(v.reshape(-1, 128).T).astype(dtype)


def pack_edges(eidx, tpc, nw, ch):
    """Partition/sort/pad one relation's edges for all cores.

    Returns (per_core list of dicts, group_windows list, e_max).
    """
    sidx_all, tidx_all = eidx[0].astype(np.int64), eidx[1].astype(np.int64)
    cores = []
    counts = np.zeros((NCORES, nw), np.int64)
    for c in range(NCORES):
        m = (tidx_all // tpc) == c
        s = sidx_all[m]
        t = tidx_all[m] - c * tpc
        o = np.argsort(t, kind='stable')
        s, t = s[o], t[o]
        cores.append((s, t))
        counts[c] = np.bincount(t // 128, minlength=nw)
    gw = np.maximum(1, np.ceil(counts.max(0) / 128).astype(np.int64))
    e_base = int(gw.sum()) * 128
    e_max = int(np.ceil(e_base / ch)) * ch
    gw[nw - 1] += (e_max - e_base) // 128

    group_windows = np.repeat(np.arange(nw), gw).astype(np.int64)
    out = []
    for c in range(NCORES):
        s, t = cores[c]
        sid = np.zeros(e_max, np.int64)
        tl = np.full(e_max, tpc, np.int64)
        msk = np.full(e_max, -1e30, np.float32)
        pos = 0
        for w in range(nw):
            n = int(counts[c, w])
            sel = slice(np.searchsorted(t, w * 128, 'left'),
                        np.searchsorted(t, (w + 1) * 128, 'left'))
            sid[pos:pos + n] = s[sel]
            tl[pos:pos + n] = t[sel]
            msk[pos:pos + n] = 0.0
            pos += int(gw[w]) * 128
        out.append({
            'sidx_w': _wrap16(sid),
            'tidx_w': _wrap16(tl),
            'tidxm': _slotmaj(tl, np.int16),
            'mask4': np.repeat(_slotmaj(msk, np.float32)[:, :, None], 4, 2),
        })
    return out, group_windows.tolist(), e_max


# ----------------------------------------------------------------------------
# device kernel builder
# ----------------------------------------------------------------------------

def build_module(cfg):
    """cfg: dict with N, TPC, NW, CH, E1, E2, GW1, GW2 (group->window lists)."""
    N, TPC, NW, CH = cfg['N'], cfg['TPC'], cfg['NW'], cfg['CH']
    nc = bacc.Bacc("TRN2")

    def din(name, shape, dt):
        return nc.dram_tensor(name, shape, dt, kind="ExternalInput")

    def dout(name, shape, dt):
        return nc.dram_tensor(name, shape, dt, kind="ExternalOutput")

    # ---- inputs: graph / features ----
    xwT = din("xwT", [128, N], BF16)           # x_wave.T
    twave_bf = din("twave_bf", [N, 128], BF16)  # t_wave
    xtransT = din("xtransT", [128, TPC], BF16)  # x_trans slice .T (per core)
    xtargT = din("xtargT", [128, TPC], BF16)    # x_target slice .T
    xtrans_r = din("xtrans_r", [TPC, 128], F32)  # residual slices, f32
    ttrans_r = din("ttrans_r", [TPC, 128], F32)
    xtarg_r = din("xtarg_r", [TPC, 128], F32)
    ttarg_r = din("ttarg_r", [TPC, 128], F32)

    edge_in = {}
    for r, E in (('1', cfg['E1']), ('2', cfg['E2'])):
        edge_in[r] = {
            'sidx_w': din(f"sidx_w{r}", [128, E // 16], I16),
            'tidx_w': din(f"tidx_w{r}", [128, E // 16], I16),
            'tidxm': din(f"tidxm{r}", [128, E // 128], I16),
            'mask4': din(f"mask4{r}", [128, (E // 128) * 4], F32),
        }

    # ---- inputs: weights (host-marshalled layouts) ----
    wconv = {}
    for r in ('1', '2'):
        wconv[r] = {
            'Wsv': din(f"Wsv{r}", [128, 512], BF16),
            'bias_s': din(f"bias_s{r}", [128, 384], F32),
            'Wtg': din(f"Wtg{r}", [128, 384], BF16),
            'bias_t': din(f"bias_t{r}", [128, 384], F32),
            'm2w': din(f"m2w{r}", [128, 512], BF16),
            'm2b': din(f"m2b{r}", [128, 4], F32),
            'WoutT': din(f"WoutT{r}", [128, 128], BF16),
            'WtT': din(f"WtT{r}", [128, 128], BF16),
            'WoutB': din(f"WoutB{r}", [128, 128], F32),
            'sw': din(f"sw{r}", [128, 1], F32),
            'fw': din(f"fw{r}", [128, 1], F32),
        }
    g_b = din("g_b", [128, 128], F32)
    b_b = din("b_b", [128, 128], F32)
    ea_b = din("ea_b", [128, 1], F32)
    rw_b = din("rw_b", [128, 1], F32)
    iota_in = din("iota_in", [128, 128], F32)

    # ---- outputs ----
    o_xtr = dout("o_xtr", [TPC, 128], F32)
    o_ttr = dout("o_ttr", [TPC, 128], F32)
    o_xtg = dout("o_xtg", [TPC, 128], F32)
    o_ttg = dout("o_ttg", [TPC, 128], F32)

    # ---- DRAM scratch ----
    S1 = nc.dram_tensor("S1", [N, 384], BF16)
    V1 = nc.dram_tensor("V1", [N, 128], BF16)
    T1 = nc.dram_tensor("T1", [TPC + 1, 384], BF16)
    S2 = nc.dram_tensor("S2", [N, 384], BF16)
    V2 = nc.dram_tensor("V2", [N, 128], BF16)
    T2 = nc.dram_tensor("T2", [TPC + 1, 384], BF16)
    xtr_bf_sl = nc.dram_tensor("xtr_bf_sl", [TPC, 128], BF16)
    ttr_bf_sl = nc.dram_tensor("ttr_bf_sl", [TPC, 128], BF16)
    xtr_bf_sh = nc.dram_tensor("xtr_bf_sh", [N, 128], BF16, addr_space="Shared")
    ttr_bf_sh = nc.dram_tensor("ttr_bf_sh", [N, 128], BF16, addr_space="Shared")
    ttr_bf = nc.dram_tensor("ttr_bf", [N, 128], BF16)
    lm_d = {r: nc.dram_tensor(f"lm_d{r}", [4, 1], F32) for r in ('1', '2')}
    ls_d = {r: nc.dram_tensor(f"ls_d{r}", [4, 1], F32) for r in ('1', '2')}
    mg_sh = {r: nc.dram_tensor(f"mg_sh{r}", [NCORES * 4, 1], F32,
                               addr_space="Shared") for r in ('1', '2')}
    zg_sh = {r: nc.dram_tensor(f"zg_sh{r}", [NCORES * 4, 1], F32,
                               addr_space="Shared") for r in ('1', '2')}

    RG = [list(range(NCORES))]
    ISQ = 1.0 / np.sqrt(HD)

    with tile.TileContext(nc) as tc:
      with ExitStack() as ctx:
        sbp = ctx.enter_context(tc.tile_pool(name="pers", bufs=1))
        sbw = ctx.enter_context(tc.tile_pool(name="work", bufs=2))
        sbn = ctx.enter_context(tc.tile_pool(name="node", bufs=3))

        # ---- constants ----
        ident_f = sbp.tile([128, 128], F32, tag="ident_f")
        make_identity(nc, ident_f[:])
        ident_b = sbp.tile([128, 128], BF16, tag="ident_b")
        nc.vector.tensor_copy(out=ident_b[:], in_=ident_f[:])
        ones_r = sbp.tile([1, 128], F32, tag="ones_r")
        nc.vector.memset(ones_r[:], 1.0)
        iota_b = sbp.tile([128, 128], F32, tag="iota")
        nc.sync.dma_start(out=iota_b[:], in_=iota_in[:])
        gb = sbp.tile([128, 128], F32, tag="gb")
        nc.sync.dma_start(out=gb[:], in_=g_b[:])
        bb = sbp.tile([128, 128], F32, tag="bb")
        nc.sync.dma_start(out=bb[:], in_=b_b[:])
        eab = sbp.tile([128, 1], F32, tag="eab")
        nc.sync.dma_start(out=eab[:], in_=ea_b[:])
        rwb = sbp.tile([128, 1], F32, tag="rwb")
        nc.sync.dma_start(out=rwb[:], in_=rw_b[:])
        epsb = sbp.tile([128, 1], F32, tag="epsb")
        nc.vector.memset(epsb[:], 1e-5)

        # ---- persistent per-conv buffers (reused by both convs) ----
        EMX = max(cfg['E1'], cfg['E2'])
        NSL = EMX // 128  # slot columns
        comb = sbp.tile([128, NSL, 4], F32, tag="comb")
        scor = sbp.tile([128, NSL, 4], F32, tag="scor")
        e4b_ = sbp.tile([128, NSL, 4], F32, tag="e4")
        tidxf = sbp.tile([128, NSL], F32, tag="tidxf")
        sidxw = sbp.tile([128, EMX // 16], I16, tag="sidxw")
        tidxw = sbp.tile([128, EMX // 16], I16, tag="tidxw")
        tidxm = sbp.tile([128, NSL], I16, tag="tidxm")
        mask4 = sbp.tile([128, NSL, 4], F32, tag="mask4")
        acc = sbp.tile([128, NW, 264], F32, tag="acc")

        def load_weights(r):
            w = {}
            for k, shp, dt in (
                    ('Wsv', [128, 512], BF16), ('bias_s', [128, 384], F32),
                    ('Wtg', [128, 384], BF16), ('bias_t', [128, 384], F32),
                    ('m2w', [128, 512], BF16), ('m2b', [128, 4], F32),
                    ('WoutT', [128, 128], BF16), ('WtT', [128, 128], BF16),
                    ('WoutB', [128, 128], F32), ('sw', [128, 1], F32),
                    ('fw', [128, 1], F32)):
                t = sbp.tile(shp, dt, tag=f"w_{k}")
                nc.sync.dma_start(out=t[:], in_=wconv[r][k][:])
                w[k] = t
            return w

        def node_phase(r, w, src_rows, S, V, T, xtT):
            """Build S/V/T projection tables. src_rows: None (use xwT input)
            or DRAM [N,128] bf16 rows to transpose on the fly."""
            psn_cm = tc.tile_pool(name="psn", bufs=2, space="PSUM")
            psn = psn_cm.__enter__()
            for n0 in range(0, N, 128):
                nw_ = min(128, N - n0)
                xT = sbn.tile([128, 128], BF16, tag="n_xT")
                if src_rows is None:
                    nc.sync.dma_start(out=xT[:, :nw_], in_=xwT[:, n0:n0 + nw_])
                else:
                    xr = sbn.tile([128, 128], BF16, tag="n_xr")
                    nc.sync.dma_start(out=xr[:nw_, :],
                                      in_=src_rows[n0:n0 + nw_, :])
                    pt = psn.tile([128, 128], BF16, tag="n_pt", space="PSUM")
                    nc.tensor.transpose(out=pt[:, :nw_], in_=xr[:nw_, :],
                                        identity=ident_b[:nw_, :nw_])
                    nc.vector.tensor_copy(out=xT[:, :nw_], in_=pt[:, :nw_])
                ps = psn.tile([128, 512], F32, tag="n_ps", space="PSUM")
                nc.tensor.matmul(ps[:nw_, :], lhsT=xT[:, :nw_], rhs=w['Wsv'][:],
                                 start=True, stop=True)
                sS = sbn.tile([128, 384], BF16, tag="n_sS")
                nc.vector.tensor_tensor(out=sS[:nw_, :], in0=ps[:nw_, 0:384],
                                        in1=w['bias_s'][:nw_, :], op=ALU.add)
                sV = sbn.tile([128, 128], BF16, tag="n_sV")
                nc.vector.tensor_copy(out=sV[:nw_, :], in_=ps[:nw_, 384:512])
                nc.sync.dma_start(out=S[n0:n0 + nw_, :], in_=sS[:nw_, :])
                nc.sync.dma_start(out=V[n0:n0 + nw_, :], in_=sV[:nw_, :])
            for n0 in range(0, TPC, 128):
                nw_ = min(128, TPC - n0)
                xT = sbn.tile([128, 128], BF16, tag="n_xT2")
                nc.sync.dma_start(out=xT[:, :nw_], in_=xtT[:, n0:n0 + nw_])
                ps = psn.tile([128, 512], F32, tag="n_ps", space="PSUM")
                nc.tensor.matmul(ps[:nw_, 0:384], lhsT=xT[:, :nw_],
                                 rhs=w['Wtg'][:], start=True, stop=True)
                sT = sbn.tile([128, 384], BF16, tag="n_sT")
                nc.vector.tensor_tensor(out=sT[:nw_, :], in0=ps[:nw_, 0:384],
                                        in1=w['bias_t'][:nw_, :], op=ALU.add)
                nc.sync.dma_start(out=T[n0:n0 + nw_, :], in_=sT[:nw_, :])
            zz = sbn.tile([1, 384], BF16, tag="n_zz")
            nc.vector.memset(zz[:], 0.0)
            nc.sync.dma_start(out=T[TPC:TPC + 1, :], in_=zz[:])
            psn_cm.__exit__(None, None, None)

        def part_reduce_to_dram(pss, src128x4, op, dst):
            """[128,4] sbuf -> partition-reduce(op) -> [4,1] -> DRAM dst."""
            pt = pss.tile([128, 128], F32, tag="pr_ps", space="PSUM")
            nc.tensor.transpose(out=pt[:4, :], in_=src128x4[:],
                                identity=ident_f[:])
            r = sbw.tile([4, 1], F32, tag="pr_r")
            nc.vector.tensor_reduce(out=r[:], in_=pt[:4, :],
                                    axis=mybir.AxisListType.X, op=op)
            nc.sync.dma_start(out=dst[:], in_=r[:])

        def combine_stats(sh_dram, op):
            """AllGather result [8*4,1] -> reduce over cores -> [1,4] sbuf."""
            t = sbw.tile([1, 32], F32, tag="cs_t")
            nc.sync.dma_start(
                out=t[:], in_=sh_dram[:].rearrange("(a b) c -> b (a c)", b=1))
            r = sbw.tile([1, 4], F32, tag="cs_r")
            nc.vector.tensor_reduce(
                out=r[:], in_=t[:].rearrange("p (c h) -> p h c", c=NCORES),
                axis=mybir.AxisListType.X, op=op)
            return r

        def bcast_part(pss, row1x4):
            """[1,4] -> [128,4] via ones matmul."""
            ps = pss.tile([128, 4], F32, tag="bc_ps", space="PSUM")
            nc.tensor.matmul(ps[:], lhsT=ones_r[:], rhs=row1x4[:],
                             start=True, stop=True)
            out = sbw.tile([128, 4], F32, tag="bc_o")
            nc.vector.tensor_copy(out=out[:], in_=ps[:])
            return out

        K_SCOPE = int(os.environ.get('K_SCOPE', '6'))

        def conv_phase(r, E, GW, src_rows, t_rows, xtT, xres, tres,
                       out_x, out_t, xbf_out, tbf_out, S, V, T):
            w = load_weights(r)
            ei = edge_in[r]
            NCH = E // CH
            NJ = CH // 128
            nsl = E // 128

            node_phase(r, w, src_rows, S, V, T, xtT)
            if K_SCOPE <= 1:
                return

            # persistent edge metadata
            nc.sync.dma_start(out=sidxw[:, :E // 16], in_=ei['sidx_w'][:])
            nc.sync.dma_start(out=tidxw[:, :E // 16], in_=ei['tidx_w'][:])
            nc.sync.dma_start(out=tidxm[:, :nsl], in_=ei['tidxm'][:])
            nc.sync.dma_start(
                out=mask4[:, :nsl, :],
                in_=ei['mask4'][:].rearrange("p (a b) -> p a b", b=4))
            nc.vector.tensor_copy(out=tidxf[:, :nsl], in_=tidxm[:, :nsl])

            GSUB = 1024  # HW limit: one dma_gather <= 1024 descriptors

            def gather(dst, table, k, elem):
                # chunk k: idxs [k*CH, (k+1)*CH) in wrapped cols [k*CH/16, ..)
                for s0 in range(0, CH, GSUB):
                    n = min(GSUB, CH - s0)
                    c0 = (k * CH + s0) // 16
                    idx = tidxw if table is T else sidxw
                    nc.gpsimd.dma_gather(
                        dst[:, s0 // 128:(s0 + n) // 128, :], table[:],
                        idx[:, c0:c0 + n // 16], n, n, elem)

            # ---- pass 1: comb + scores ----
            for k in range(NCH):
                j0 = k * NJ
                Sg = sbw.tile([128, NJ, 384], BF16, tag="p1_Sg")
                gather(Sg, S, k, 384)
                Tg = sbw.tile([128, NJ, 384], BF16, tag="p1_Tg")
                gather(Tg, T, k, 384)
                # mlp: relu(hs+ht)
                hr = sbw.tile([128, NJ, 128], BF16, tag="p1_hr")
                nc.vector.tensor_tensor(out=hr[:], in0=Sg[:, :, 0:128],
                                        in1=Tg[:, :, 0:128], op=ALU.add)
                nc.scalar.activation(hr[:], hr[:], AF.Relu)
                mw = sbw.tile([128, NJ, 4], F32, tag="p1_mw")
                qk = sbw.tile([128, NJ, 128], BF16, tag="p1_qk")
                tmp = qk
                for h4 in range(4):
                    nc.vector.tensor_tensor(
                        out=tmp[:], in0=hr[:],
                        in1=w['m2w'][:, h4 * 128:(h4 + 1) * 128][:, None, :]
                        .to_broadcast([128, NJ, 128]),
                        op=ALU.mult)
                    nc.vector.tensor_reduce(out=mw[:, :, h4:h4 + 1],
                                            in_=tmp[:],
                                            axis=mybir.AxisListType.X,
                                            op=ALU.add)
                nc.vector.tensor_tensor(
                    out=mw[:], in0=mw[:],
                    in1=w['m2b'][:, None, :].to_broadcast([128, NJ, 4]),
                    op=ALU.add)
                nc.scalar.activation(mw[:], mw[:], AF.Sigmoid)
                # attn
                nc.vector.tensor_tensor(out=qk[:], in0=Sg[:, :, 128:256],
                                        in1=Tg[:, :, 128:256], op=ALU.mult)
                at = sbw.tile([128, NJ, 4], F32, tag="p1_at")
                nc.vector.tensor_reduce(
                    out=at[:], in_=qk[:].rearrange("p a (b c) -> p a b c", b=4),
                    axis=mybir.AxisListType.X, op=ALU.add)
                nc.scalar.activation(at[:], at[:], AF.Sigmoid, scale=ISQ)
                # comb = sw*mw + fw*at + mask
                cb = comb[:, j0:j0 + NJ, :]
                nc.vector.tensor_scalar(out=cb, in0=mw[:], scalar1=w['sw'][:],
                                        scalar2=None, op0=ALU.mult)
                nc.vector.scalar_tensor_tensor(out=cb, in0=at[:],
                                               scalar=w['fw'][:], in1=cb,
                                               op0=ALU.mult, op1=ALU.add)
                nc.vector.tensor_tensor(out=cb, in0=cb,
                                        in1=mask4[:, j0:j0 + NJ, :],
                                        op=ALU.add)
                # scores
                nc.vector.tensor_tensor(out=qk[:], in0=Sg[:, :, 256:384],
                                        in1=Tg[:, :, 256:384], op=ALU.mult)
                sc = scor[:, j0:j0 + NJ, :]
                nc.vector.tensor_reduce(
                    out=sc, in_=qk[:].rearrange("p a (b c) -> p a b c", b=4),
                    axis=mybir.AxisListType.X, op=ALU.add)
                nc.scalar.activation(sc, sc, AF.Copy, scale=ISQ)

            # ---- global softmax stats ----
            if K_SCOPE <= 2:
                return
            pss_cm = tc.tile_pool(name="pss", bufs=1, space="PSUM")
            pss = pss_cm.__enter__()
            cmax = sbw.tile([128, 4], F32, tag="st_cmax")
            nc.vector.tensor_reduce(
                out=cmax[:],
                in_=comb[:, 0:nsl, :].rearrange("p a b -> p b a"),
                axis=mybir.AxisListType.X, op=ALU.max)
            part_reduce_to_dram(pss, cmax, ALU.max, lm_d[r])
            nc.gpsimd.collective_compute(
                "AllGather", ALU.bypass, replica_groups=RG,
                ins=[lm_d[r][:]], outs=[mg_sh[r][:]])
            mg = combine_stats(mg_sh[r], ALU.max)        # [1,4]
            mgb = bcast_part(pss, mg)                          # [128,4]
            # ec = exp(comb - Mg) (in place)
            nc.vector.tensor_tensor(
                out=comb[:, 0:nsl, :], in0=comb[:, 0:nsl, :],
                in1=mgb[:, None, :].to_broadcast([128, nsl, 4]),
                op=ALU.subtract)
            nc.scalar.activation(comb[:, 0:nsl, :], comb[:, 0:nsl, :], AF.Exp)
            esum = sbw.tile([128, 4], F32, tag="st_esum")
            nc.vector.tensor_reduce(
                out=esum[:],
                in_=comb[:, 0:nsl, :].rearrange("p a b -> p b a"),
                axis=mybir.AxisListType.X, op=ALU.add)
            part_reduce_to_dram(pss, esum, ALU.add, ls_d[r])
            nc.gpsimd.collective_compute(
                "AllGather", ALU.bypass, replica_groups=RG,
                ins=[ls_d[r][:]], outs=[zg_sh[r][:]])
            zg = combine_stats(zg_sh[r], ALU.add)
            zi = sbw.tile([1, 4], F32, tag="st_zi")
            nc.vector.reciprocal(zi[:], zg[:])
            zib = bcast_part(pss, zi)
            # w = scores * ec * zi ; e4 = exp(w)
            nc.vector.tensor_tensor(out=e4b_[:, 0:nsl, :],
                                    in0=scor[:, 0:nsl, :],
                                    in1=comb[:, 0:nsl, :], op=ALU.mult)
            nc.vector.tensor_tensor(
                out=e4b_[:, 0:nsl, :], in0=e4b_[:, 0:nsl, :],
                in1=zib[:, None, :].to_broadcast([128, nsl, 4]),
                op=ALU.mult)
            nc.scalar.activation(e4b_[:, 0:nsl, :], e4b_[:, 0:nsl, :], AF.Exp)
            pss_cm.__exit__(None, None, None)
            if K_SCOPE <= 3:
                return

            # ---- pass 3: gather V/t, one-hot matmul aggregation ----
            # window -> first/last group
            first_g = {}
            last_g = {}
            for g, ww in enumerate(GW):
                first_g.setdefault(ww, g)
                last_g[ww] = g
            psa_cm = tc.tile_pool(name="psa", bufs=2, space="PSUM")
            psa = psa_cm.__enter__()
            pacc = {}
            for k in range(NCH):
                Vg = sbw.tile([128, NJ, 128], BF16, tag="p3_Vg")
                gather(Vg, V, k, 128)
                tg = sbw.tile([128, NJ, 128], BF16, tag="p3_tg")
                gather(tg, t_rows, k, 128)
                P = sbw.tile([128, NJ, 264], BF16, tag="p3_P")
                e4c = sbw.tile([128, NJ, 4], BF16, tag="p3_e4c")
                nc.vector.tensor_copy(out=e4c[:], in_=e4b_[:, k * NJ:(k + 1) * NJ, :])
                nc.vector.tensor_tensor(
                    out=P[:, :, 0:128].rearrange("p a (b c) -> p a b c", b=4),
                    in0=Vg[:].rearrange("p a (b c) -> p a b c", b=4),
                    in1=e4c[:, :, :, None].to_broadcast([128, NJ, 4, 32]),
                    op=ALU.mult)
                nc.vector.tensor_copy(out=P[:, :, 128:256], in_=tg[:])
                nc.vector.tensor_copy(out=P[:, :, 256:260], in_=e4c[:])
                nc.vector.memset(P[:, :, 260:261], 1.0)
                nc.vector.memset(P[:, :, 261:264], 0.0)
                for j in range(NJ):
                    g = k * NJ + j
                    ww = GW[g]
                    ts = sbw.tile([128, 1], F32, tag="p3_ts")
                    nc.vector.tensor_scalar(
                        out=ts[:], in0=tidxf[:, g:g + 1],
                        scalar1=float(ww * 128), scalar2=None,
                        op0=ALU.subtract)
                    oh = sbw.tile([128, 128], BF16, tag="p3_oh")
                    nc.vector.tensor_tensor(
                        out=oh[:], in0=ts[:].to_broadcast([128, 128]),
                        in1=iota_b[:], op=ALU.is_equal)
                    if ww not in pacc:
                        pacc[ww] = psa.tile([128, 264], F32, tag="p3_acc",
                                            name=f"p3_acc_{ww}", space="PSUM")
                    nc.tensor.matmul(pacc[ww][:], lhsT=oh[:], rhs=P[:, j, :],
                                     start=(g == first_g[ww]),
                                     stop=(g == last_g[ww]))
                    if g == last_g[ww]:
                        nc.vector.tensor_copy(out=acc[:, ww, :],
                                              in_=pacc.pop(ww)[:])
            for ww in range(NW):
                if ww not in first_g:
                    nc.vector.memset(acc[:, ww, :], 0.0)
            psa_cm.__exit__(None, None, None)
            if K_SCOPE <= 4:
                return

            # ---- finalize per window ----
            psf_cm = tc.tile_pool(name="psf", bufs=1, space="PSUM")
            psf = psf_cm.__enter__()
            for ww in range(NW):
                vw = min(TPC, (ww + 1) * 128) - ww * 128
                if vw <= 0:
                    continue
                sl = acc[:, ww, :]
                dsafe = sbw.tile([128, 4], F32, tag="f_ds")
                nc.vector.tensor_scalar(out=dsafe[:], in0=sl[:, 256:260],
                                        scalar1=1e-16, scalar2=None,
                                        op0=ALU.max)
                rd = sbw.tile([128, 4], F32, tag="f_rd")
                nc.vector.reciprocal(rd[:], dsafe[:])
                xw_ = sbw.tile([128, 128], BF16, tag="f_xw")
                nc.vector.tensor_tensor(
                    out=xw_[:].rearrange("p (b c) -> p b c", b=4),
                    in0=sl[:, 0:128].rearrange("p (b c) -> p b c", b=4),
                    in1=rd[:, :, None].to_broadcast([128, 4, 32]),
                    op=ALU.mult)
                tac = sbw.tile([128, 128], BF16, tag="f_ta")
                nc.vector.tensor_copy(out=tac[:], in_=sl[:, 128:256])

                def proj(rows_bf, W, tagp):
                    ptr = psf.tile([128, 128], BF16, tag=f"f_ptr{tagp}",
                                   space="PSUM")
                    nc.tensor.transpose(out=ptr[:], in_=rows_bf[:],
                                        identity=ident_b[:])
                    sT = sbw.tile([128, 128], BF16, tag=f"f_sT{tagp}")
                    nc.vector.tensor_copy(out=sT[:], in_=ptr[:])
                    pm = psf.tile([128, 128], F32, tag=f"f_pm{tagp}",
                                  space="PSUM")
                    nc.tensor.matmul(pm[:], lhsT=sT[:], rhs=W[:],
                                     start=True, stop=True)
                    return pm

                pm_x = proj(xw_, w['WoutT'], "x")
                pm_t = proj(tac, w['WtT'], "t")

                def ln_out(pm, bias_deg, res_dram, out_dram, bf_out):
                    y = sbw.tile([128, 128], F32, tag="f_y")
                    if bias_deg is not None:
                        nc.vector.scalar_tensor_tensor(
                            out=y[:], in0=w['WoutB'][:], scalar=bias_deg,
                            in1=pm[:], op0=ALU.mult, op1=ALU.add)
                        nc.scalar.activation(y[:], y[:], AF.Relu)
                    else:
                        nc.scalar.activation(y[:], pm[:], AF.Relu)
                    res = sbw.tile([128, 128], F32, tag="f_res")
                    if vw < 128:
                        nc.vector.memset(res[:], 0.0)
                    nc.sync.dma_start(out=res[:vw, :],
                                      in_=res_dram[ww * 128:ww * 128 + vw, :])
                    nc.vector.tensor_scalar(out=res[:], in0=res[:],
                                            scalar1=rwb[:], scalar2=None,
                                            op0=ALU.mult)
                    nc.vector.scalar_tensor_tensor(out=y[:], in0=y[:],
                                                   scalar=eab[:], in1=res[:],
                                                   op0=ALU.mult, op1=ALU.add)
                    # layernorm over free dim
                    m = sbw.tile([128, 1], F32, tag="f_m")
                    nc.vector.tensor_reduce(out=m[:], in_=y[:],
                                            axis=mybir.AxisListType.X,
                                            op=ALU.add)
                    nc.vector.tensor_scalar(out=m[:], in0=m[:],
                                            scalar1=1.0 / 128, scalar2=None,
                                            op0=ALU.mult)
                    nc.vector.tensor_scalar(out=y[:], in0=y[:], scalar1=m[:],
                                            scalar2=None, op0=ALU.subtract)
                    sq = sbw.tile([128, 128], F32, tag="f_sq")
                    nc.scalar.activation(sq[:], y[:], AF.Square)
                    v_ = sbw.tile([128, 1], F32, tag="f_v")
                    nc.vector.tensor_reduce(out=v_[:], in_=sq[:],
                                            axis=mybir.AxisListType.X,
                                            op=ALU.add)
                    nc.scalar.activation(v_[:], v_[:], AF.Sqrt,
                                         scale=1.0 / 128, bias=epsb[:])
                    rstd = sbw.tile([128, 1], F32, tag="f_rstd")
                    nc.vector.reciprocal(rstd[:], v_[:])
                    nc.vector.tensor_scalar(out=y[:], in0=y[:], scalar1=rstd[:],
                                            scalar2=None, op0=ALU.mult)
                    nc.vector.tensor_tensor(out=y[:], in0=y[:], in1=gb[:],
                                            op=ALU.mult)
                    nc.vector.tensor_tensor(out=y[:], in0=y[:], in1=bb[:],
                                            op=ALU.add)
                    nc.sync.dma_start(out=out_dram[ww * 128:ww * 128 + vw, :],
                                      in_=y[:vw, :])
                    if bf_out is not None:
                        yb = sbw.tile([128, 128], BF16, tag="f_yb")
                        nc.vector.tensor_copy(out=yb[:], in_=y[:])
                        nc.sync.dma_start(
                            out=bf_out[ww * 128:ww * 128 + vw, :],
                            in_=yb[:vw, :])

                ln_out(pm_x, sl[:, 260:261], xres, out_x, xbf_out)
                ln_out(pm_t, None, tres, out_t, tbf_out)
            psf_cm.__exit__(None, None, None)

        # ================= conv1 =================
        conv_phase('1', cfg['E1'], cfg['GW1'], None, twave_bf, xtransT,
                   xtrans_r, ttrans_r, o_xtr, o_ttr, xtr_bf_sl, ttr_bf_sl,
                   S1, V1, T1)

        # allgather updated trans features
        if K_SCOPE <= 5:
            nc.finalize = nc.finalize  # no-op marker
        nc.gpsimd.collective_compute("AllGather", ALU.bypass,
                                     replica_groups=RG,
                                     ins=[xtr_bf_sl[:]], outs=[xtr_bf_sh[:]])
        nc.gpsimd.collective_compute("AllGather", ALU.bypass,
                                     replica_groups=RG,
                                     ins=[ttr_bf_sl[:]], outs=[ttr_bf_sh[:]])
        nc.sync.dma_start(out=ttr_bf[:], in_=ttr_bf_sh[:])

        # ================= conv2 =================
        if K_SCOPE >= 6:
            conv_phase('2', cfg['E2'], cfg['GW2'], xtr_bf_sh, ttr_bf,
                       xtargT, xtarg_r, ttarg_r, o_xtg, o_ttg, None, None,
                       S2, V2, T2)

    nc.finalize()
    return nc


# ----------------------------------------------------------------------------
# host wrapper
# ----------------------------------------------------------------------------

def _marshal_conv_weights(p):
    bf = lambda a: np.ascontiguousarray(a).astype(BF16NP)
    f3 = lambda a: np.ascontiguousarray(a).astype(np.float32)
    mlp1 = np.asarray(p['mlp1_w'], np.float32)
    out = {
        'Wsv': bf(np.concatenate([mlp1[:, :128].T, np.asarray(p['q_w']).T,
                                  np.asarray(p['Wk']).T,
                                  np.asarray(p['Wv']).T], 1)),
        'Wtg': bf(np.concatenate([mlp1[:, 128:].T, np.asarray(p['k_w']).T,
                                  np.asarray(p['Wq']).T], 1)),
        'm2w': bf(np.concatenate(
            [np.broadcast_to(np.asarray(p['mlp2_w'])[h], (128, 128))
             for h in range(4)], 1)),
        'm2b': f3(np.broadcast_to(np.asarray(p['mlp2_b']), (128, 4))),
        'WoutT': bf(np.asarray(p['Wout_w']).T),
        'WtT': bf(np.asarray(p['Wt']).T),
        'WoutB': f3(np.broadcast_to(np.asarray(p['Wout_b']), (128, 128))),
        'sw': f3(np.broadcast_to(np.asarray(p['sw']).reshape(1, 1), (128, 1))),
        'fw': f3(np.broadcast_to(np.asarray(p['fw']).reshape(1, 1), (128, 1))),
    }
    bias_s = np.zeros((128, 384), np.float32)
    bias_s[:, 128:256] = np.asarray(p['q_b'], np.float32)
    bias_t = np.zeros((128, 384), np.float32)
    bias_t[:, 0:128] = np.asarray(p['mlp1_b'], np.float32)
    bias_t[:, 128:256] = np.asarray(p['k_b'], np.float32)
    out['bias_s'] = bias_s
    out['bias_t'] = bias_t
    return out


def run(inputs, N, CH=2048, run_fn=None):
    """Build + run the SPMD kernel on 8 cores. run_fn(nc, in_maps) may be
    injected (simulation); default = run_bass_kernel_spmd."""
    TPC = N // NCORES
    NW = (TPC + 1 + 127) // 128
    params = inputs['params']
    bf = lambda a: np.ascontiguousarray(a).astype(BF16NP)
    f3 = lambda a: np.ascontiguousarray(np.asarray(a, np.float32))

    e1, GW1, E1 = pack_edges(np.asarray(inputs['edge_index_wt']), TPC, NW, CH)
    e2, GW2, E2 = pack_edges(np.asarray(inputs['edge_index_tt']), TPC, NW, CH)

    cfg = {'N': N, 'TPC': TPC, 'NW': NW, 'CH': CH,
           'E1': E1, 'E2': E2, 'GW1': GW1, 'GW2': GW2}
    nc = build_module(cfg)

    w1 = _marshal_conv_weights(params['conv1'])
    w2 = _marshal_conv_weights(params['conv2'])
    com = {
        'xwT': bf(np.asarray(inputs['x_wave']).T),
        'twave_bf': bf(inputs['t_wave']),
        'g_b': f3(np.broadcast_to(np.asarray(params['ln_gamma']), (128, 128))),
        'b_b': f3(np.broadcast_to(np.asarray(params['ln_beta']), (128, 128))),
        'ea_b': f3(np.broadcast_to(
            np.asarray(params['edge_weight_adaptation']).reshape(1, 1),
            (128, 1))),
        'rw_b': f3(np.broadcast_to(
            np.asarray(params['residual_weight']).reshape(1, 1), (128, 1))),
        'iota_in': np.broadcast_to(
            np.arange(128, dtype=np.float32), (128, 128)).copy(),
    }
    for k, v in w1.items():
        com[f"{k}1"] = v
    for k, v in w2.items():
        com[f"{k}2"] = v

    in_maps = []
    for c in range(NCORES):
        s = slice(c * TPC, (c + 1) * TPC)
        m = dict(com)
        m['xtransT'] = bf(np.asarray(inputs['x_trans'])[s].T)
        m['xtargT'] = bf(np.asarray(inputs['x_target'])[s].T)
        m['xtrans_r'] = f3(np.asarray(inputs['x_trans'])[s])
        m['ttrans_r'] = f3(np.asarray(inputs['t_trans'])[s])
        m['xtarg_r'] = f3(np.asarray(inputs['x_target'])[s])
        m['ttarg_r'] = f3(np.asarray(inputs['t_target'])[s])
        for r, ee in (('1', e1), ('2', e2)):
            m[f'sidx_w{r}'] = ee[c]['sidx_w']
            m[f'tidx_w{r}'] = ee[c]['tidx_w']
            m[f'tidxm{r}'] = ee[c]['tidxm']
            m[f'mask4{r}'] = ee[c]['mask4'].reshape(128, -1)
        in_maps.append(m)

    if run_fn is None:
        from concourse.bass_utils import run_bass_kernel_spmd
        res = run_bass_kernel_spmd(nc, in_maps, list(range(NCORES)))
        results = res.results
    else:
        results = run_fn(nc, in_maps)

    x_tr = np.concatenate([results[c]['o_xtr'] for c in range(NCORES)], 0)
    t_tr = np.concatenate([results[c]['o_ttr'] for c in range(NCORES)], 0)
    x_tg = np.concatenate([results[c]['o_xtg'] for c in range(NCORES)], 0)
    t_tg = np.concatenate([results[c]['o_ttg'] for c in range(NCORES)], 0)
    return np.stack([x_tr.reshape(N, 128), x_tg.reshape(N, 128),
                     t_tr.reshape(N, 128), t_tg.reshape(N, 128)])


def kernel(**inputs) -> np.ndarray:
    return run(inputs, N=20000).astype(np.float32)
